# revision 2
# baseline (speedup 1.0000x reference)
"""GQA kernel for 8 NeuronCores (TRN2, Bass/Tile).

Sharding: core c = (batch b = c//4, kv-group g = c%4).  Each core computes
q-heads [4g,4g+4) and kv-head g for batch b, applies RoPE + causal attention
+ its 512-row slice of the o-projection, producing a partial [2048,2048]
output.  Host sums the 4 partials per batch.

Layout trick: all matmuls contract over the partition dim, so we ship x
pre-transposed (xT = x[b].T) and compute Q^T,K^T in [d,s] layout and V in
[s,d] layout directly.  Scores are computed transposed (S^T[k,q]) so the
attention-value product and o-projection need no on-device transposes.
Softmax runs without max subtraction (scores are O(+-6)); the denominator
comes from a ones-vector matmul and is applied to the PSUM attention output
during its copy to SBUF.

Scheduling: projections are ordered K, q0, V, q1..q3 with RoPE emitted as
soon as each head's raw projection lands, so head-0 attention overlaps the
tail projections and the PE never idles long enough to re-throttle (HAM).
The attention inner loop is software-pipelined two deep: the scores matmul
for step j+2 issues before the exp-gated EV/Z matmuls of step j.
"""
import math

import numpy as np
import ml_dtypes

import concourse.bass as bass
import concourse.bacc as bacc
import concourse.mybir as mybir
import concourse.tile as tile
from concourse.bass_utils import run_bass_kernel_spmd

BF16 = mybir.dt.bfloat16
F32 = mybir.dt.float32

DIM = 2048
S = 2048
HD = 128          # head dim
NH = 4            # q heads per core
DHC = NH * HD     # 512: per-core o-proj contraction
KT16 = DIM // 128  # 16 contraction tiles
ST16 = S // 128    # 16 seq tiles
NC_CHUNK = 512     # q-chunk width / matmul free dim
NCH = S // NC_CHUNK  # 4 q-chunks
SCALE = 1.0 / math.sqrt(HD)
ROPE_BASE = 10000.0


def _rope_tables():
    inv_freq = 1.0 / (ROPE_BASE ** (np.arange(0, HD, 2, dtype=np.float64) / HD))
    t = np.arange(S, dtype=np.float64)
    freqs = np.outer(t, inv_freq)                      # [S, 64]
    emb = np.concatenate([freqs, freqs], axis=1)       # [S, 128]
    cosT = np.cos(emb).T.astype(np.float32)            # [128, S]
    sinT = np.sin(emb).T.astype(np.float32)
    # fold rotate-half sign into sin: rope = t*cos + shift(t)*sT
    # shift(t)[0:64]=t[64:128], shift(t)[64:128]=t[0:64]
    sT = sinT.copy()
    sT[0:64] = -sT[0:64]
    return cosT, sT


def _diag_masks():
    # mask[kk, o*512+qq] = 1 if (o*128 + kk) <= qq else 0, o in 0..3
    kk = np.arange(128)[:, None]
    qq = np.arange(NC_CHUNK)[None, :]
    cols = [((o * 128 + kk) <= qq).astype(np.float32) for o in range(4)]
    return np.concatenate(cols, axis=1)                # [128, 2048]


def build_nc():
    nc = bacc.Bacc("TRN2", target_bir_lowering=False, debug=False)
    xt_d = nc.dram_tensor("xt", [DIM, S], BF16, kind="ExternalInput")
    wq_d = nc.dram_tensor("wq", [DIM, DHC], BF16, kind="ExternalInput")
    wk_d = nc.dram_tensor("wk", [DIM, HD], BF16, kind="ExternalInput")
    wv_d = nc.dram_tensor("wv", [DIM, HD], BF16, kind="ExternalInput")
    wo_d = nc.dram_tensor("wo", [DHC, DIM], BF16, kind="ExternalInput")
    out_d = nc.dram_tensor("out", [S, DIM], F32, kind="ExternalOutput")

    cosT_np, sT_np = _rope_tables()
    cos_h = nc.inline_tensor(cosT_np.astype(ml_dtypes.bfloat16), name="cosT")
    sin_h = nc.inline_tensor(sT_np.astype(ml_dtypes.bfloat16), name="sinT")
    mask_h = nc.inline_tensor(_diag_masks().astype(ml_dtypes.bfloat16), name="masks")

    Exp = mybir.ActivationFunctionType.Exp
    MUL = mybir.AluOpType.mult

    with tile.TileContext(nc) as tc:
        with tc.tile_pool(name="constp", bufs=1) as constp, \
             tc.tile_pool(name="p_qkv", bufs=1) as p_qkv, \
             tc.tile_pool(name="p_ot", bufs=1) as p_ot:
            wo_sb = constp.tile([128, NH * DIM], BF16)
            for h in range(NH):
                nc.sync.dma_start(out=wo_sb[:, h * DIM:(h + 1) * DIM],
                                  in_=wo_d.ap()[h * 128:(h + 1) * 128, :])
            masks_sb = constp.tile([128, 2048], BF16)
            nc.sync.dma_start(out=masks_sb[:], in_=mask_h.ap())
            ones_sb = constp.tile([128, 1], BF16)
            nc.vector.memset(ones_sb[:], 1.0)

            cos_sb = p_qkv.tile([128, S], BF16)
            nc.sync.dma_start(out=cos_sb[:], in_=cos_h.ap())
            sin_sb = p_qkv.tile([128, S], BF16)
            nc.sync.dma_start(out=sin_sb[:], in_=sin_h.ap())

            # rope is applied in place: qt/kt are the projection outputs
            qt = [p_qkv.tile([128, S], BF16, name=f"qt{h}") for h in range(NH)]
            kt = p_qkv.tile([128, S], BF16)
            v_sb = p_qkv.tile([128, ST16 * HD], BF16)
            ot = [p_ot.tile([128, S], BF16, name=f"ot{h}") for h in range(NH)]

            with tc.tile_pool(name="p1", bufs=1) as p1, \
                 tc.tile_pool(name="p_att", bufs=1) as p_att, \
                 tc.tile_pool(name="ps", bufs=1, space="PSUM") as ps:
                # ---- HAM warmup: race the input DMAs with dummy matmuls ---
                warm_sb = p1.tile([128, NC_CHUNK], BF16)
                nc.vector.memset(warm_sb[:], 1.0)
                for i in range(16):
                    w_ps = ps.tile([128, NC_CHUNK], F32, tag="qps", bufs=2)
                    nc.tensor.matmul(w_ps[:], warm_sb[:, 0:128], warm_sb[:],
                                     start=True, stop=True)

                # ---- loads ----
                xt_sb = p1.tile([128, KT16 * S], BF16)
                for k in range(KT16):
                    nc.sync.dma_start(out=xt_sb[:, k * S:(k + 1) * S],
                                      in_=xt_d.ap()[k * 128:(k + 1) * 128, :])
                wq_sb = p1.tile([128, KT16 * DHC], BF16)
                wk_sb = p1.tile([128, KT16 * HD], BF16)
                wv_sb = p1.tile([128, KT16 * HD], BF16)
                for k in range(KT16):
                    nc.sync.dma_start(out=wk_sb[:, k * HD:(k + 1) * HD],
                                      in_=wk_d.ap()[k * 128:(k + 1) * 128, :])
                    nc.sync.dma_start(out=wq_sb[:, k * DHC:(k + 1) * DHC],
                                      in_=wq_d.ap()[k * 128:(k + 1) * 128, :])
                    nc.sync.dma_start(out=wv_sb[:, k * HD:(k + 1) * HD],
                                      in_=wv_d.ap()[k * 128:(k + 1) * 128, :])

                def proj_qk(h):
                    # h in 0..NH-1 -> q head h; h == NH -> K
                    dst = qt[h] if h < NH else kt
                    for n in range(NCH):
                        q_ps = ps.tile([128, NC_CHUNK], F32, tag="qps", bufs=2)
                        for k in range(KT16):
                            if h < NH:
                                lhsT = wq_sb[:, k * DHC + h * HD:
                                             k * DHC + (h + 1) * HD]
                            else:
                                lhsT = wk_sb[:, k * HD:(k + 1) * HD]
                            nc.tensor.matmul(
                                q_ps[:], lhsT,
                                xt_sb[:, k * S + n * NC_CHUNK:
                                      k * S + (n + 1) * NC_CHUNK],
                                start=(k == 0), stop=(k == KT16 - 1))
                        nc.scalar.copy(dst[:, n * NC_CHUNK:(n + 1) * NC_CHUNK],
                                       q_ps[:])

                def rope(h):
                    # in-place: qt/kt currently hold the raw projection
                    t = qt[h] if h < NH else kt
                    shf = p1.tile([128, S], BF16, tag="shift")
                    nc.vector.tensor_copy(shf[0:64, :], t[64:128, :])
                    nc.vector.tensor_copy(shf[64:128, :], t[0:64, :])
                    m1 = p1.tile([128, S], BF16, tag="ropetmp")
                    nc.vector.tensor_tensor(m1[:], t[:], cos_sb[:], MUL)
                    m2 = p1.tile([128, S], BF16, tag="ropetmp2")
                    nc.vector.tensor_tensor(m2[:], shf[:], sin_sb[:], MUL)
                    nc.vector.tensor_add(t[:], m1[:], m2[:])

                def proj_v():
                    for t in range(ST16):
                        v_ps = ps.tile([128, HD], F32, tag="qps", bufs=2)
                        for k in range(KT16):
                            nc.tensor.matmul(
                                v_ps[:],
                                xt_sb[:, k * S + t * 128:k * S + (t + 1) * 128],
                                wv_sb[:, k * HD:(k + 1) * HD],
                                start=(k == 0), stop=(k == KT16 - 1))
                        nc.scalar.copy(v_sb[:, t * HD:(t + 1) * HD], v_ps[:])

                def attention(h):
                    for c in range(NCH):
                        nk = 4 * c + 4
                        o_ps = ps.tile([128, NC_CHUNK], F32, tag="ops", bufs=2)
                        z_ps = ps.tile([1, NC_CHUNK], F32, tag="zps", bufs=1)
                        pend = []
                        for j in range(nk):
                            s_ps = ps.tile([128, NC_CHUNK], F32, tag="sps",
                                           bufs=3)
                            nc.tensor.matmul(
                                s_ps[:], kt[:, j * 128:(j + 1) * 128],
                                qt[h][:, c * NC_CHUNK:(c + 1) * NC_CHUNK],
                                start=True, stop=True)
                            e_t = p_att.tile([128, NC_CHUNK], BF16, tag="e",
                                             bufs=6)
                            nc.scalar.activation(e_t[:], s_ps[:], Exp,
                                                 scale=SCALE)
                            o = j - 4 * c
                            if o >= 0:
                                nc.vector.tensor_tensor(
                                    e_t[:], e_t[:],
                                    masks_sb[:, o * NC_CHUNK:(o + 1) * NC_CHUNK],
                                    MUL)
                            pend.append((j, e_t))
                            if len(pend) > 2:
                                pj, pe = pend.pop(0)
                                nc.tensor.matmul(
                                    o_ps[:], v_sb[:, pj * HD:(pj + 1) * HD],
                                    pe[:], start=(pj == 0), stop=False)
                                nc.tensor.matmul(z_ps[:], ones_sb[:], pe[:],
                                                 start=(pj == 0), stop=False)
                        for pj, pe in pend:
                            last = pj == nk - 1
                            nc.tensor.matmul(o_ps[:],
                                             v_sb[:, pj * HD:(pj + 1) * HD],
                                             pe[:], start=(pj == 0), stop=last)
                            nc.tensor.matmul(z_ps[:], ones_sb[:], pe[:],
                                             start=(pj == 0), stop=last)

                        zsb = p_att.tile([1, NC_CHUNK], F32, tag="zsb", bufs=2)
                        nc.vector.tensor_copy(zsb[:], z_ps[:])
                        zr = p_att.tile([1, NC_CHUNK], F32, tag="zr", bufs=2)
                        nc.vector.reciprocal_approx_fast(out=zr[:], in_=zsb[:])
                        rb = p_att.tile([128, NC_CHUNK], F32, tag="rb", bufs=2)
                        nc.gpsimd.partition_broadcast(rb[:], zr[:])
                        nc.vector.tensor_tensor(
                            ot[h][:, c * NC_CHUNK:(c + 1) * NC_CHUNK],
                            o_ps[:], rb[:], MUL)

                # emission order: K and q0 projections first, their ropes,
                # then V; head-0 attention then overlaps q1..q3 projections.
                proj_qk(NH)   # K
                proj_qk(0)
                rope(NH)
                rope(0)
                proj_v()
                attention(0)
                for h in range(1, NH):
                    proj_qk(h)
                    rope(h)
                    attention(h)

            # ---------------- phase 3: o-projection -----------------------
            with tc.tile_pool(name="p_out", bufs=4) as p_out, \
                 tc.tile_pool(name="ps_o", bufs=2, space="PSUM") as ps_o:
                for t in range(ST16):
                    for n in range(NCH):
                        out_ps = ps_o.tile([128, NC_CHUNK], F32)
                        for h in range(NH):
                            nc.tensor.matmul(
                                out_ps[:], ot[h][:, t * 128:(t + 1) * 128],
                                wo_sb[:, h * DIM + n * NC_CHUNK:
                                      h * DIM + (n + 1) * NC_CHUNK],
                                start=(h == 0), stop=(h == NH - 1))
                        out_sb = p_out.tile([128, NC_CHUNK], F32)
                        if (t * NCH + n) % 2 == 0:
                            nc.scalar.copy(out_sb[:], out_ps[:])
                        else:
                            nc.vector.tensor_copy(out_sb[:], out_ps[:])
                        nc.sync.dma_start(
                            out=out_d.ap()[t * 128:(t + 1) * 128,
                                           n * NC_CHUNK:(n + 1) * NC_CHUNK],
                            in_=out_sb[:])
    nc.compile()
    return nc


_NC_CACHE = []


def kernel(x, wq, wk, wv, wo):
    if not _NC_CACHE:
        _NC_CACHE.append(build_nc())
    nc = _NC_CACHE[0]
    bf = ml_dtypes.bfloat16
    xT = np.ascontiguousarray(x.transpose(0, 2, 1)).astype(bf)   # [B, DIM, S]
    in_maps = []
    for c in range(8):
        b, g = c // 4, c % 4
        in_maps.append({
            "xt": xT[b],
            "wq": np.ascontiguousarray(wq[:, g * DHC:(g + 1) * DHC]).astype(bf),
            "wk": np.ascontiguousarray(wk[:, g * HD:(g + 1) * HD]).astype(bf),
            "wv": np.ascontiguousarray(wv[:, g * HD:(g + 1) * HD]).astype(bf),
            "wo": np.ascontiguousarray(wo[g * DHC:(g + 1) * DHC, :]).astype(bf),
        })
    res = run_bass_kernel_spmd(nc, in_maps, list(range(8)))
    out = np.zeros((2, S, DIM), np.float32)
    for c in range(8):
        out[c // 4] += res.results[c]["out"]
    return out


# revision 5
# speedup vs baseline: 1.0309x; 1.0309x over previous
"""GQA kernel for 8 NeuronCores (TRN2, Bass/Tile).

Sharding: core c = (batch b = c//4, kv-group g = c%4).  Each core computes
q-heads [4g,4g+4) and kv-head g for batch b, applies RoPE + causal attention
+ its 512-row slice of the o-projection, producing a partial [2048,2048]
output.  Host sums the 4 partials per batch.

Layout trick: all matmuls contract over the partition dim, so we ship x
pre-transposed (xT = x[b].T) and compute Q^T,K^T in [d,s] layout and V in
[s,d] layout directly.  Scores are computed transposed (S^T[k,q]) so the
attention-value product and o-projection need no on-device transposes.
Softmax runs without max subtraction (scores are O(+-6)); the denominator
comes from a ones-vector matmul and is applied to the PSUM attention output
during its copy to SBUF.

Scheduling: projections are ordered K, q0, V, q1..q3 with RoPE emitted as
soon as each head's raw projection lands, so head-0 attention overlaps the
tail projections and the PE never idles long enough to re-throttle (HAM).
The attention inner loop is software-pipelined two deep: the scores matmul
for step j+2 issues before the exp-gated EV/Z matmuls of step j.
"""
import math

import numpy as np
import ml_dtypes

import concourse.bass as bass
import concourse.bacc as bacc
import concourse.mybir as mybir
import concourse.tile as tile
from concourse.bass_utils import run_bass_kernel_spmd

BF16 = mybir.dt.bfloat16
F32 = mybir.dt.float32

DIM = 2048
S = 2048
HD = 128          # head dim
NH = 4            # q heads per core
DHC = NH * HD     # 512: per-core o-proj contraction
KT16 = DIM // 128  # 16 contraction tiles
ST16 = S // 128    # 16 seq tiles
NC_CHUNK = 512     # q-chunk width / matmul free dim
NCH = S // NC_CHUNK  # 4 q-chunks
SCALE = 1.0 / math.sqrt(HD)
ROPE_BASE = 10000.0


def _rope_tables():
    inv_freq = 1.0 / (ROPE_BASE ** (np.arange(0, HD, 2, dtype=np.float64) / HD))
    t = np.arange(S, dtype=np.float64)
    freqs = np.outer(t, inv_freq)                      # [S, 64]
    emb = np.concatenate([freqs, freqs], axis=1)       # [S, 128]
    cosT = np.cos(emb).T.astype(np.float32)            # [128, S]
    sinT = np.sin(emb).T.astype(np.float32)
    # fold rotate-half sign into sin: rope = t*cos + shift(t)*sT
    # shift(t)[0:64]=t[64:128], shift(t)[64:128]=t[0:64]
    sT = sinT.copy()
    sT[0:64] = -sT[0:64]
    return cosT, sT


def _diag_masks():
    # mask[kk, o*512+qq] = 1 if (o*128 + kk) <= qq else 0, o in 0..3
    kk = np.arange(128)[:, None]
    qq = np.arange(NC_CHUNK)[None, :]
    cols = [((o * 128 + kk) <= qq).astype(np.float32) for o in range(4)]
    return np.concatenate(cols, axis=1)                # [128, 2048]


def build_nc():
    nc = bacc.Bacc("TRN2", target_bir_lowering=False, debug=False)
    xt_d = nc.dram_tensor("xt", [DIM, S], BF16, kind="ExternalInput")
    wq_d = nc.dram_tensor("wq", [DIM, DHC], BF16, kind="ExternalInput")
    wk_d = nc.dram_tensor("wk", [DIM, HD], BF16, kind="ExternalInput")
    wv_d = nc.dram_tensor("wv", [DIM, HD], BF16, kind="ExternalInput")
    wo_d = nc.dram_tensor("wo", [DHC, DIM], BF16, kind="ExternalInput")
    out_d = nc.dram_tensor("out", [S, DIM], F32, kind="ExternalOutput")

    cosT_np, sT_np = _rope_tables()
    cos_h = nc.inline_tensor(cosT_np.astype(ml_dtypes.bfloat16), name="cosT")
    sin_h = nc.inline_tensor(sT_np.astype(ml_dtypes.bfloat16), name="sinT")
    mask_h = nc.inline_tensor(_diag_masks().astype(ml_dtypes.bfloat16), name="masks")

    Exp = mybir.ActivationFunctionType.Exp
    MUL = mybir.AluOpType.mult

    with tile.TileContext(nc) as tc:
        with tc.tile_pool(name="constp", bufs=1) as constp, \
             tc.tile_pool(name="p_qkv", bufs=1) as p_qkv, \
             tc.tile_pool(name="p_ot", bufs=1) as p_ot:
            wo_sb = constp.tile([128, NH * DIM], BF16)
            masks_sb = constp.tile([128, 2048], BF16)
            ones_sb = constp.tile([128, 1], BF16)
            nc.vector.memset(ones_sb[:], 1.0)
            cos_sb = p_qkv.tile([128, S], BF16)
            sin_sb = p_qkv.tile([128, S], BF16)

            # rope is applied in place: qt/kt are the projection outputs
            qt = [p_qkv.tile([128, S], BF16, name=f"qt{h}") for h in range(NH)]
            kt = p_qkv.tile([128, S], BF16)
            v_sb = p_qkv.tile([128, ST16 * HD], BF16)
            ot = [p_ot.tile([128, S], BF16, name=f"ot{h}") for h in range(NH)]

            with tc.tile_pool(name="p1", bufs=1) as p1, \
                 tc.tile_pool(name="p_att", bufs=1) as p_att, \
                 tc.tile_pool(name="ps", bufs=1, space="PSUM") as ps:
                # ---- HAM warmup: race the input DMAs with dummy matmuls ---
                warm_sb = p1.tile([128, NC_CHUNK], BF16)
                nc.vector.memset(warm_sb[:], 1.0)
                for i in range(16):
                    w_ps = ps.tile([128, NC_CHUNK], F32, tag="qps", bufs=2)
                    nc.tensor.matmul(w_ps[:], warm_sb[:, 0:128], warm_sb[:],
                                     start=True, stop=True)

                # ---- loads: small weights first so projection chains can
                # start as soon as the first xt tiles land; wo/masks (not
                # needed until attention / phase 3) queue after xt.
                xt_sb = p1.tile([128, KT16 * S], BF16)
                wq_sb = p1.tile([128, KT16 * DHC], BF16)
                wk_sb = p1.tile([128, KT16 * HD], BF16)
                wv_sb = p1.tile([128, KT16 * HD], BF16)
                for k in range(KT16):
                    nc.sync.dma_start(out=wk_sb[:, k * HD:(k + 1) * HD],
                                      in_=wk_d.ap()[k * 128:(k + 1) * 128, :])
                    nc.sync.dma_start(out=wv_sb[:, k * HD:(k + 1) * HD],
                                      in_=wv_d.ap()[k * 128:(k + 1) * 128, :])
                for k in range(KT16):
                    nc.sync.dma_start(out=wq_sb[:, k * DHC:(k + 1) * DHC],
                                      in_=wq_d.ap()[k * 128:(k + 1) * 128, :])
                nc.sync.dma_start(out=cos_sb[:], in_=cos_h.ap())
                nc.sync.dma_start(out=sin_sb[:], in_=sin_h.ap())
                for k in range(KT16):
                    nc.sync.dma_start(out=xt_sb[:, k * S:(k + 1) * S],
                                      in_=xt_d.ap()[k * 128:(k + 1) * 128, :])
                nc.sync.dma_start(out=masks_sb[:], in_=mask_h.ap())
                for h in range(NH):
                    nc.sync.dma_start(out=wo_sb[:, h * DIM:(h + 1) * DIM],
                                      in_=wo_d.ap()[h * 128:(h + 1) * 128, :])

                def proj_qk(h):
                    # h in 0..NH-1 -> q head h; h == NH -> K
                    dst = qt[h] if h < NH else kt
                    for n in range(NCH):
                        q_ps = ps.tile([128, NC_CHUNK], F32, tag="qps", bufs=2)
                        for k in range(KT16):
                            if h < NH:
                                lhsT = wq_sb[:, k * DHC + h * HD:
                                             k * DHC + (h + 1) * HD]
                            else:
                                lhsT = wk_sb[:, k * HD:(k + 1) * HD]
                            nc.tensor.matmul(
                                q_ps[:], lhsT,
                                xt_sb[:, k * S + n * NC_CHUNK:
                                      k * S + (n + 1) * NC_CHUNK],
                                start=(k == 0), stop=(k == KT16 - 1))
                        nc.scalar.copy(dst[:, n * NC_CHUNK:(n + 1) * NC_CHUNK],
                                       q_ps[:])

                def rope(h):
                    # in-place: qt/kt currently hold the raw projection
                    t = qt[h] if h < NH else kt
                    shf = p1.tile([128, S], BF16, tag="shift")
                    nc.vector.tensor_copy(shf[0:64, :], t[64:128, :])
                    nc.vector.tensor_copy(shf[64:128, :], t[0:64, :])
                    m1 = p1.tile([128, S], BF16, tag="ropetmp")
                    nc.vector.tensor_tensor(m1[:], t[:], cos_sb[:], MUL)
                    m2 = p1.tile([128, S], BF16, tag="ropetmp2")
                    nc.vector.tensor_tensor(m2[:], shf[:], sin_sb[:], MUL)
                    nc.vector.tensor_add(t[:], m1[:], m2[:])

                def proj_v():
                    for t in range(ST16):
                        v_ps = ps.tile([128, HD], F32, tag="qps", bufs=2)
                        for k in range(KT16):
                            nc.tensor.matmul(
                                v_ps[:],
                                xt_sb[:, k * S + t * 128:k * S + (t + 1) * 128],
                                wv_sb[:, k * HD:(k + 1) * HD],
                                start=(k == 0), stop=(k == KT16 - 1))
                        nc.scalar.copy(v_sb[:, t * HD:(t + 1) * HD], v_ps[:])

                def attention(h):
                    for c in range(NCH):
                        nk = 4 * c + 4
                        o_ps = ps.tile([128, NC_CHUNK], F32, tag="ops", bufs=2)
                        z_ps = ps.tile([1, NC_CHUNK], F32, tag="zps", bufs=1)
                        pend = []
                        for j in range(nk):
                            s_ps = ps.tile([128, NC_CHUNK], F32, tag="sps",
                                           bufs=3)
                            nc.tensor.matmul(
                                s_ps[:], kt[:, j * 128:(j + 1) * 128],
                                qt[h][:, c * NC_CHUNK:(c + 1) * NC_CHUNK],
                                start=True, stop=True)
                            e_t = p_att.tile([128, NC_CHUNK], BF16, tag="e",
                                             bufs=6)
                            nc.scalar.activation(e_t[:], s_ps[:], Exp,
                                                 scale=SCALE)
                            o = j - 4 * c
                            if o >= 0:
                                nc.vector.tensor_tensor(
                                    e_t[:], e_t[:],
                                    masks_sb[:, o * NC_CHUNK:(o + 1) * NC_CHUNK],
                                    MUL)
                            pend.append((j, e_t))
                            if len(pend) > 2:
                                pj, pe = pend.pop(0)
                                nc.tensor.matmul(
                                    o_ps[:], v_sb[:, pj * HD:(pj + 1) * HD],
                                    pe[:], start=(pj == 0), stop=False)
                                nc.tensor.matmul(z_ps[:], ones_sb[:], pe[:],
                                                 start=(pj == 0), stop=False)
                        for pj, pe in pend:
                            last = pj == nk - 1
                            nc.tensor.matmul(o_ps[:],
                                             v_sb[:, pj * HD:(pj + 1) * HD],
                                             pe[:], start=(pj == 0), stop=last)
                            nc.tensor.matmul(z_ps[:], ones_sb[:], pe[:],
                                             start=(pj == 0), stop=last)

                        zsb = p_att.tile([1, NC_CHUNK], F32, tag="zsb", bufs=2)
                        nc.vector.tensor_copy(zsb[:], z_ps[:])
                        zr = p_att.tile([1, NC_CHUNK], F32, tag="zr", bufs=2)
                        nc.vector.reciprocal_approx_fast(out=zr[:], in_=zsb[:])
                        rb = p_att.tile([128, NC_CHUNK], F32, tag="rb", bufs=2)
                        nc.gpsimd.partition_broadcast(rb[:], zr[:])
                        nc.vector.tensor_tensor(
                            ot[h][:, c * NC_CHUNK:(c + 1) * NC_CHUNK],
                            o_ps[:], rb[:], MUL)

                # emission order: all projections back-to-back on the PE
                # (K first so its rope starts earliest), ropes interleaved on
                # the DVE as each head's raw projection lands; by the time
                # the PE reaches the attention stream every rope is done.
                proj_qk(NH)   # K
                proj_qk(0)
                rope(NH)
                rope(0)
                proj_v()
                for h in range(1, NH):
                    proj_qk(h)
                    rope(h)
                for h in range(NH):
                    attention(h)

            # ---------------- phase 3: o-projection -----------------------
            with tc.tile_pool(name="p_out", bufs=4) as p_out, \
                 tc.tile_pool(name="ps_o", bufs=2, space="PSUM") as ps_o:
                for t in range(ST16):
                    for n in range(NCH):
                        out_ps = ps_o.tile([128, NC_CHUNK], F32)
                        for h in range(NH):
                            nc.tensor.matmul(
                                out_ps[:], ot[h][:, t * 128:(t + 1) * 128],
                                wo_sb[:, h * DIM + n * NC_CHUNK:
                                      h * DIM + (n + 1) * NC_CHUNK],
                                start=(h == 0), stop=(h == NH - 1))
                        out_sb = p_out.tile([128, NC_CHUNK], F32)
                        if (t * NCH + n) % 2 == 0:
                            nc.scalar.copy(out_sb[:], out_ps[:])
                        else:
                            nc.vector.tensor_copy(out_sb[:], out_ps[:])
                        nc.sync.dma_start(
                            out=out_d.ap()[t * 128:(t + 1) * 128,
                                           n * NC_CHUNK:(n + 1) * NC_CHUNK],
                            in_=out_sb[:])
    nc.compile()
    return nc


_NC_CACHE = []


def kernel(x, wq, wk, wv, wo):
    if not _NC_CACHE:
        _NC_CACHE.append(build_nc())
    nc = _NC_CACHE[0]
    bf = ml_dtypes.bfloat16
    xT = np.ascontiguousarray(x.transpose(0, 2, 1)).astype(bf)   # [B, DIM, S]
    in_maps = []
    for c in range(8):
        b, g = c // 4, c % 4
        in_maps.append({
            "xt": xT[b],
            "wq": np.ascontiguousarray(wq[:, g * DHC:(g + 1) * DHC]).astype(bf),
            "wk": np.ascontiguousarray(wk[:, g * HD:(g + 1) * HD]).astype(bf),
            "wv": np.ascontiguousarray(wv[:, g * HD:(g + 1) * HD]).astype(bf),
            "wo": np.ascontiguousarray(wo[g * DHC:(g + 1) * DHC, :]).astype(bf),
        })
    res = run_bass_kernel_spmd(nc, in_maps, list(range(8)))
    out = np.zeros((2, S, DIM), np.float32)
    for c in range(8):
        out[c // 4] += res.results[c]["out"]
    return out


# revision 6
# speedup vs baseline: 1.1040x; 1.0709x over previous
"""GQA kernel for 8 NeuronCores (TRN2, Bass/Tile).

Sharding: core c = (batch b = c//4, kv-group g = c%4).  Each core computes
q-heads [4g,4g+4) and kv-head g for batch b, applies RoPE + causal attention
+ its 512-row slice of the o-projection, producing a partial [2048,2048]
output.  Host sums the 4 partials per batch.

Layout trick: all matmuls contract over the partition dim, so we ship x
pre-transposed (xT = x[b].T) and compute Q^T,K^T in [d,s] layout and V in
[s,d] layout directly.  Scores are computed transposed (S^T[k,q]) so the
attention-value product and o-projection need no on-device transposes.
Softmax runs without max subtraction (scores are O(+-6)); the denominator
comes from a ones-vector matmul and is applied to the PSUM attention output
during its copy to SBUF.

Scheduling: projections are ordered K, q0, V, q1..q3 with RoPE emitted as
soon as each head's raw projection lands, so head-0 attention overlaps the
tail projections and the PE never idles long enough to re-throttle (HAM).
The attention inner loop is software-pipelined two deep: the scores matmul
for step j+2 issues before the exp-gated EV/Z matmuls of step j.
"""
import math

import numpy as np
import ml_dtypes

import concourse.bass as bass
import concourse.bacc as bacc
import concourse.mybir as mybir
import concourse.tile as tile
from concourse.bass_utils import run_bass_kernel_spmd

BF16 = mybir.dt.bfloat16
F32 = mybir.dt.float32

DIM = 2048
S = 2048
HD = 128          # head dim
NH = 4            # q heads per core
DHC = NH * HD     # 512: per-core o-proj contraction
KT16 = DIM // 128  # 16 contraction tiles
ST16 = S // 128    # 16 seq tiles
NC_CHUNK = 512     # q-chunk width / matmul free dim
NCH = S // NC_CHUNK  # 4 q-chunks
SCALE = 1.0 / math.sqrt(HD)
ROPE_BASE = 10000.0


def _rope_tables():
    inv_freq = 1.0 / (ROPE_BASE ** (np.arange(0, HD, 2, dtype=np.float64) / HD))
    t = np.arange(S, dtype=np.float64)
    freqs = np.outer(t, inv_freq)                      # [S, 64]
    emb = np.concatenate([freqs, freqs], axis=1)       # [S, 128]
    cosT = np.cos(emb).T.astype(np.float32)            # [128, S]
    sinT = np.sin(emb).T.astype(np.float32)
    # fold rotate-half sign into sin: rope = t*cos + shift(t)*sT
    # shift(t)[0:64]=t[64:128], shift(t)[64:128]=t[0:64]
    sT = sinT.copy()
    sT[0:64] = -sT[0:64]
    return cosT, sT


def _diag_masks():
    # mask[kk, o*512+qq] = 1 if (o*128 + kk) <= qq else 0, o in 0..3
    kk = np.arange(128)[:, None]
    qq = np.arange(NC_CHUNK)[None, :]
    cols = [((o * 128 + kk) <= qq).astype(np.float32) for o in range(4)]
    return np.concatenate(cols, axis=1)                # [128, 2048]


def build_nc():
    nc = bacc.Bacc("TRN2", target_bir_lowering=False, debug=False)
    xt_d = nc.dram_tensor("xt", [DIM, S], BF16, kind="ExternalInput")
    wq_d = nc.dram_tensor("wq", [DIM, DHC], BF16, kind="ExternalInput")
    wk_d = nc.dram_tensor("wk", [DIM, HD], BF16, kind="ExternalInput")
    wv_d = nc.dram_tensor("wv", [DIM, HD], BF16, kind="ExternalInput")
    wo_d = nc.dram_tensor("wo", [DHC, DIM], BF16, kind="ExternalInput")
    out_d = nc.dram_tensor("out", [S, DIM], F32, kind="ExternalOutput")

    cosT_np, sT_np = _rope_tables()
    cos_h = nc.inline_tensor(cosT_np.astype(ml_dtypes.bfloat16), name="cosT")
    sin_h = nc.inline_tensor(sT_np.astype(ml_dtypes.bfloat16), name="sinT")
    mask_h = nc.inline_tensor(_diag_masks().astype(ml_dtypes.bfloat16), name="masks")

    Exp = mybir.ActivationFunctionType.Exp
    MUL = mybir.AluOpType.mult

    with tile.TileContext(nc) as tc:
        with tc.tile_pool(name="constp", bufs=1) as constp, \
             tc.tile_pool(name="p_qkv", bufs=1) as p_qkv, \
             tc.tile_pool(name="p_ot", bufs=1) as p_ot:
            wo_sb = constp.tile([128, NH * DIM], BF16)
            masks_sb = constp.tile([128, 2048], BF16)
            ones_sb = constp.tile([128, 1], BF16)
            nc.vector.memset(ones_sb[:], 1.0)
            cos_sb = p_qkv.tile([128, S], BF16)
            sin_sb = p_qkv.tile([128, S], BF16)

            # rope is applied in place: qt/kt are the projection outputs
            qt = [p_qkv.tile([128, S], BF16, name=f"qt{h}") for h in range(NH)]
            kt = p_qkv.tile([128, S], BF16)
            v_sb = p_qkv.tile([128, ST16 * HD], BF16)
            ot = [p_ot.tile([128, S], BF16, name=f"ot{h}") for h in range(NH)]

            with tc.tile_pool(name="p1", bufs=1) as p1, \
                 tc.tile_pool(name="p_att", bufs=1) as p_att, \
                 tc.tile_pool(name="ps", bufs=1, space="PSUM") as ps:
                # ---- HAM warmup: race the input DMAs with dummy matmuls ---
                warm_sb = p1.tile([128, NC_CHUNK], BF16)
                nc.vector.memset(warm_sb[:], 1.0)
                for i in range(16):
                    w_ps = ps.tile([128, NC_CHUNK], F32, tag="qps", bufs=2)
                    nc.tensor.matmul(w_ps[:], warm_sb[:, 0:128], warm_sb[:],
                                     start=True, stop=True)

                # ---- loads: small weights first so projection chains can
                # start as soon as the first xt tiles land; wo/masks (not
                # needed until attention / phase 3) queue after xt.
                xt_sb = p1.tile([128, KT16 * S], BF16)
                wq_sb = p1.tile([128, KT16 * DHC], BF16)
                wk_sb = p1.tile([128, KT16 * HD], BF16)
                wv_sb = p1.tile([128, KT16 * HD], BF16)
                for k in range(KT16):
                    nc.sync.dma_start(out=wk_sb[:, k * HD:(k + 1) * HD],
                                      in_=wk_d.ap()[k * 128:(k + 1) * 128, :])
                    nc.sync.dma_start(out=wv_sb[:, k * HD:(k + 1) * HD],
                                      in_=wv_d.ap()[k * 128:(k + 1) * 128, :])
                for k in range(KT16):
                    nc.sync.dma_start(out=wq_sb[:, k * DHC:(k + 1) * DHC],
                                      in_=wq_d.ap()[k * 128:(k + 1) * 128, :])
                nc.sync.dma_start(out=cos_sb[:], in_=cos_h.ap())
                nc.sync.dma_start(out=sin_sb[:], in_=sin_h.ap())
                # xt goes through SWDGE (static, in-order queues) so early
                # k-tiles complete before late ones and the projection
                # chains can start consuming them mid-load.
                for k in range(KT16):
                    nc.gpsimd.dma_start(out=xt_sb[:, k * S:(k + 1) * S],
                                        in_=xt_d.ap()[k * 128:(k + 1) * 128, :])
                nc.sync.dma_start(out=masks_sb[:], in_=mask_h.ap())
                for h in range(NH):
                    nc.sync.dma_start(out=wo_sb[:, h * DIM:(h + 1) * DIM],
                                      in_=wo_d.ap()[h * 128:(h + 1) * 128, :])

                def proj_qk(h):
                    # h in 0..NH-1 -> q head h; h == NH -> K
                    dst = qt[h] if h < NH else kt
                    for n in range(NCH):
                        q_ps = ps.tile([128, NC_CHUNK], F32, tag="qps", bufs=2)
                        for k in range(KT16):
                            if h < NH:
                                lhsT = wq_sb[:, k * DHC + h * HD:
                                             k * DHC + (h + 1) * HD]
                            else:
                                lhsT = wk_sb[:, k * HD:(k + 1) * HD]
                            nc.tensor.matmul(
                                q_ps[:], lhsT,
                                xt_sb[:, k * S + n * NC_CHUNK:
                                      k * S + (n + 1) * NC_CHUNK],
                                start=(k == 0), stop=(k == KT16 - 1))
                        nc.scalar.copy(dst[:, n * NC_CHUNK:(n + 1) * NC_CHUNK],
                                       q_ps[:])

                def rope(h):
                    # in-place: qt/kt currently hold the raw projection
                    t = qt[h] if h < NH else kt
                    shf = p1.tile([128, S], BF16, tag="shift")
                    nc.vector.tensor_copy(shf[0:64, :], t[64:128, :])
                    nc.vector.tensor_copy(shf[64:128, :], t[0:64, :])
                    m1 = p1.tile([128, S], BF16, tag="ropetmp")
                    nc.vector.tensor_tensor(m1[:], t[:], cos_sb[:], MUL)
                    m2 = p1.tile([128, S], BF16, tag="ropetmp2")
                    nc.vector.tensor_tensor(m2[:], shf[:], sin_sb[:], MUL)
                    nc.vector.tensor_add(t[:], m1[:], m2[:])

                def proj_v():
                    for t in range(ST16):
                        v_ps = ps.tile([128, HD], F32, tag="qps", bufs=2)
                        for k in range(KT16):
                            nc.tensor.matmul(
                                v_ps[:],
                                xt_sb[:, k * S + t * 128:k * S + (t + 1) * 128],
                                wv_sb[:, k * HD:(k + 1) * HD],
                                start=(k == 0), stop=(k == KT16 - 1))
                        nc.scalar.copy(v_sb[:, t * HD:(t + 1) * HD], v_ps[:])

                def attention(h):
                    for c in range(NCH):
                        nk = 4 * c + 4
                        o_ps = ps.tile([128, NC_CHUNK], F32, tag="ops", bufs=2)
                        z_ps = ps.tile([1, NC_CHUNK], F32, tag="zps", bufs=1)
                        pend = []
                        for j in range(nk):
                            s_ps = ps.tile([128, NC_CHUNK], F32, tag="sps",
                                           bufs=3)
                            nc.tensor.matmul(
                                s_ps[:], kt[:, j * 128:(j + 1) * 128],
                                qt[h][:, c * NC_CHUNK:(c + 1) * NC_CHUNK],
                                start=True, stop=True)
                            e_t = p_att.tile([128, NC_CHUNK], BF16, tag="e",
                                             bufs=6)
                            nc.scalar.activation(e_t[:], s_ps[:], Exp,
                                                 scale=SCALE)
                            o = j - 4 * c
                            if o >= 0:
                                nc.vector.tensor_tensor(
                                    e_t[:], e_t[:],
                                    masks_sb[:, o * NC_CHUNK:(o + 1) * NC_CHUNK],
                                    MUL)
                            pend.append((j, e_t))
                            if len(pend) > 2:
                                pj, pe = pend.pop(0)
                                nc.tensor.matmul(
                                    o_ps[:], v_sb[:, pj * HD:(pj + 1) * HD],
                                    pe[:], start=(pj == 0), stop=False)
                                nc.tensor.matmul(z_ps[:], ones_sb[:], pe[:],
                                                 start=(pj == 0), stop=False)
                        for pj, pe in pend:
                            last = pj == nk - 1
                            nc.tensor.matmul(o_ps[:],
                                             v_sb[:, pj * HD:(pj + 1) * HD],
                                             pe[:], start=(pj == 0), stop=last)
                            nc.tensor.matmul(z_ps[:], ones_sb[:], pe[:],
                                             start=(pj == 0), stop=last)

                        zsb = p_att.tile([1, NC_CHUNK], F32, tag="zsb", bufs=2)
                        nc.vector.tensor_copy(zsb[:], z_ps[:])
                        zr = p_att.tile([1, NC_CHUNK], F32, tag="zr", bufs=2)
                        nc.vector.reciprocal_approx_fast(out=zr[:], in_=zsb[:])
                        rb = p_att.tile([128, NC_CHUNK], F32, tag="rb", bufs=2)
                        nc.gpsimd.partition_broadcast(rb[:], zr[:])
                        nc.vector.tensor_tensor(
                            ot[h][:, c * NC_CHUNK:(c + 1) * NC_CHUNK],
                            o_ps[:], rb[:], MUL)

                # emission order: all projections back-to-back on the PE
                # (K first so its rope starts earliest), ropes interleaved on
                # the DVE as each head's raw projection lands; by the time
                # the PE reaches the attention stream every rope is done.
                proj_qk(NH)   # K
                proj_qk(0)
                rope(NH)
                rope(0)
                proj_v()
                for h in range(1, NH):
                    proj_qk(h)
                    rope(h)
                for h in range(NH):
                    attention(h)

            # ---------------- phase 3: o-projection -----------------------
            with tc.tile_pool(name="p_out", bufs=4) as p_out, \
                 tc.tile_pool(name="ps_o", bufs=2, space="PSUM") as ps_o:
                for t in range(ST16):
                    for n in range(NCH):
                        out_ps = ps_o.tile([128, NC_CHUNK], F32)
                        for h in range(NH):
                            nc.tensor.matmul(
                                out_ps[:], ot[h][:, t * 128:(t + 1) * 128],
                                wo_sb[:, h * DIM + n * NC_CHUNK:
                                      h * DIM + (n + 1) * NC_CHUNK],
                                start=(h == 0), stop=(h == NH - 1))
                        out_sb = p_out.tile([128, NC_CHUNK], F32)
                        if (t * NCH + n) % 2 == 0:
                            nc.scalar.copy(out_sb[:], out_ps[:])
                        else:
                            nc.vector.tensor_copy(out_sb[:], out_ps[:])
                        nc.sync.dma_start(
                            out=out_d.ap()[t * 128:(t + 1) * 128,
                                           n * NC_CHUNK:(n + 1) * NC_CHUNK],
                            in_=out_sb[:])
    nc.compile()
    return nc


_NC_CACHE = []


def kernel(x, wq, wk, wv, wo):
    if not _NC_CACHE:
        _NC_CACHE.append(build_nc())
    nc = _NC_CACHE[0]
    bf = ml_dtypes.bfloat16
    xT = np.ascontiguousarray(x.transpose(0, 2, 1)).astype(bf)   # [B, DIM, S]
    in_maps = []
    for c in range(8):
        b, g = c // 4, c % 4
        in_maps.append({
            "xt": xT[b],
            "wq": np.ascontiguousarray(wq[:, g * DHC:(g + 1) * DHC]).astype(bf),
            "wk": np.ascontiguousarray(wk[:, g * HD:(g + 1) * HD]).astype(bf),
            "wv": np.ascontiguousarray(wv[:, g * HD:(g + 1) * HD]).astype(bf),
            "wo": np.ascontiguousarray(wo[g * DHC:(g + 1) * DHC, :]).astype(bf),
        })
    res = run_bass_kernel_spmd(nc, in_maps, list(range(8)))
    out = np.zeros((2, S, DIM), np.float32)
    for c in range(8):
        out[c // 4] += res.results[c]["out"]
    return out
